# revision 6
# baseline (speedup 1.0000x reference)
"""Trainium2 Bass kernel for nn_ContrastLossLocal (supervised-contrastive loss).

Strategy (8 NeuronCores, SPMD via run_bass_kernel_spmd):
  - Host sorts anchors by class label and pads each class block to a
    multiple of 128 rows; total padded tiles rounded to a multiple of 8.
  - Launch A (anchor tiles sharded over cores): dots of each core's anchor
    tiles vs all 608 prototypes (pure fp32 matmul), shipped to host.
    Host derives logits_max (masked row max, exact) from these dots.
  - Launch B (local memory sharded 8-way over the N=2048 axis, anchors
    replicated): per class, matmul of that class's anchor tiles vs the
    core's 256-row local shard, exp(dots - logits_max) with on-ACT
    accumulation -> per-anchor partial negative sums; also the row max of
    the exp values so the host can restore IEEE inf semantics even if the
    ACT table saturates on overflow.
  - Host: sums partials across cores, fixes overflow rows to +inf, and
    evaluates the final [4096, 32] log-prob/masked-mean in float32 numpy
    (bit-faithful to the reference's where/exp/log/0*inf semantics).

Every instruction is arranged to carry at most ONE semaphore wait (this
toolchain's walrus rejects multi-wait instructions): single blob load DMA,
single-engine writers per DMA'd tile, PSUM read by exactly one engine,
no scratch-slot reuse on the ACT/DVE pipe.
"""
import numpy as np

import concourse.bass as bass
import concourse.tile as tile
import concourse.mybir as mybir
from concourse.bass_utils import run_bass_kernel_spmd

F32 = mybir.dt.float32
AF = mybir.ActivationFunctionType
AX = mybir.AxisListType


def _install_drain_split():
    """This container's walrus rejects instructions with >1 semaphore wait.
    Tile's kernel-tail drain collects one wait per processor; split the
    surplus onto extra drain instructions (same guarantee, 1 wait each)."""
    from concourse.vector_clock import ScopedClock

    def patched(self, tick_clock, wait_clock):
        drain_inst = self.nc.sync.drain()
        wait_clock.add_sem_waits(
            drain_inst.ins, ScopedClock({None: tick_clock.global_clock}))
        si = drain_inst.ins.sync_info
        if si is not None:
            waits = list(si.on_wait)
            if len(waits) > 1:
                drain_inst.ins.sync_info = mybir.SyncInfo(
                    on_wait=[waits[0]], on_update=list(si.on_update))
                for w in waits[1:]:
                    extra = self.nc.sync.drain()
                    extra.ins.sync_info = mybir.SyncInfo(
                        on_wait=[w], on_update=[])
        self.nc.all_engine_barrier()
        assert self.sems is not None
        popped = self.nc._tile_sem_poison_stack.pop()
        assert popped is self._sem_poison
        self.nc.clear_and_free_semaphores(list(self.sems.allocated().values()))
        self.nc.all_engine_barrier()

    tile.TileContext._drain_and_barrier = patched


_install_drain_split()

NCORES = 8
C, U, N, D = 19, 32, 2048, 256
A = 4096
P = C * U            # 608
NSH = N // NCORES    # 256 local rows per class per core
TEMP = np.float32(0.1)
EPS = np.float32(1e-8)
FMAX = np.float32(np.finfo(np.float32).max)


# ---------------------------------------------------------------- launch A
def build_phase_a(tpc):
    """Per core: its tpc anchor tiles x all 608 protos -> raw dots out."""
    nc = bass.Bass()
    ncols = 2 * tpc * 128 + 2 * P            # astk0|astk1|ptk0|ptk1
    blob = nc.declare_dram_parameter("blob", [128, ncols], F32, isOutput=False)
    dots = nc.declare_dram_parameter("dots", [128, tpc * P], F32, isOutput=True)

    a0, a1 = 0, tpc * 128
    p0, p1 = 2 * tpc * 128, 2 * tpc * 128 + P

    with tile.TileContext(nc) as tc:
        with tc.tile_pool(name="sbuf", bufs=1) as pool, \
             tc.tile_pool(name="psum", bufs=2, space="PSUM") as pp:
            sb = pool.tile([128, ncols], F32)
            nc.sync.dma_start(sb[:], blob[:])
            out = pool.tile([128, tpc * P], F32)

            for t in range(tpc):
                psA = pp.tile([128, 320], F32, tag="psA")
                psB = pp.tile([128, 288], F32, tag="psB")
                for k, ak in ((0, a0), (1, a1)):
                    lhsT = sb[:, ak + t * 128: ak + (t + 1) * 128]
                    pk = p0 if k == 0 else p1
                    nc.tensor.matmul(psA[:], lhsT, sb[:, pk: pk + 320],
                                     start=(k == 0), stop=(k == 1))
                    nc.tensor.matmul(psB[:], lhsT, sb[:, pk + 320: pk + P],
                                     start=(k == 0), stop=(k == 1))
                nc.vector.tensor_scalar_add(out[:, t * P: t * P + 320],
                                            psA[:], 0.0)
                nc.vector.tensor_scalar_add(out[:, t * P + 320: (t + 1) * P],
                                            psB[:], 0.0)
            nc.sync.dma_start(dots[:], out[:])
    return nc


# ---------------------------------------------------------------- launch B
def build_phase_b(T, class_of_tile):
    """All T anchor tiles x this core's local shard (per-class 256 rows)."""
    nc = bass.Bass()
    At = T * 128
    ncols = 2 * At + 2 * C * NSH + T         # astk0|astk1|lstk0|lstk1|mneg
    blob = nc.declare_dram_parameter("blob", [128, ncols], F32, isOutput=False)
    outp = nc.declare_dram_parameter("out", [128, 2 * T], F32, isOutput=True)

    a0, a1 = 0, At
    l0, l1 = 2 * At, 2 * At + C * NSH
    mcol = 2 * At + 2 * C * NSH

    with tile.TileContext(nc) as tc:
        with tc.tile_pool(name="sbuf", bufs=1) as pool, \
             tc.tile_pool(name="psum", bufs=4, space="PSUM") as pp:
            sb = pool.tile([128, ncols], F32)
            nc.sync.dma_start(sb[:], blob[:])

            # ACT absorber: observe the blob DMA once so later exps only
            # ever need the single PE wait (walrus allows 1 wait/instr).
            absorb = pool.tile([128, 1], F32)
            nc.scalar.copy(absorb[:], sb[:, mcol: mcol + 1])

            ebig = pool.tile([128, T * NSH], F32)     # exp scratch, no reuse
            negp = pool.tile([128, T], F32)           # ACT accum sums
            emax = pool.tile([128, T], F32)           # DVE row maxes
            outb = pool.tile([128, 2 * T], F32)       # DVE-written store buf

            for t in range(T):
                c = class_of_tile[t]
                ps = pp.tile([128, NSH], F32, tag="ps")
                for k, (ak, lk) in enumerate(((a0, l0), (a1, l1))):
                    nc.tensor.matmul(
                        ps[:],
                        sb[:, ak + t * 128: ak + (t + 1) * 128],
                        sb[:, lk + c * NSH: lk + (c + 1) * NSH],
                        start=(k == 0), stop=(k == 1))
                esl = ebig[:, t * NSH: (t + 1) * NSH]
                nc.scalar.activation(esl, ps[:], AF.Exp,
                                     bias=sb[:, mcol + t: mcol + t + 1],
                                     accum_out=negp[:, t: t + 1])
                nc.vector.reduce_max(emax[:, t: t + 1], esl, axis=AX.X)

            nc.vector.tensor_scalar_add(outb[:, :T], negp[:], 0.0)
            nc.vector.tensor_scalar_add(outb[:, T:], emax[:], 0.0)
            nc.sync.dma_start(outp[:], outb[:])
    return nc


# ---------------------------------------------------------------- host glue
def _run(nc, in_maps, profile):
    """Run the SPMD program; if profiling, re-run warm and record wall ns."""
    import time
    res = run_bass_kernel_spmd(nc, in_maps, list(range(NCORES)), trace=False)
    if profile is not None:
        t0 = time.perf_counter()
        run_bass_kernel_spmd(nc, in_maps, list(range(NCORES)), trace=False)
        profile.append(int((time.perf_counter() - t0) * 1e9))
    return res


def kernel(anchors, anchor_labels, proto_mem, proto_mask, local_mem,
           _profile=None):
    anchors = np.asarray(anchors, np.float32)
    labels = np.asarray(anchor_labels, np.int32)
    proto_mem = np.asarray(proto_mem, np.float32)
    proto_mask = np.asarray(proto_mask, np.int32)
    local_mem = np.asarray(local_mem, np.float32)

    # ---- sort/pad anchors by class -------------------------------------
    counts = np.bincount(labels, minlength=C)
    tiles_c = [int(np.ceil(n / 128)) if n else 0 for n in counts]
    T_raw = sum(tiles_c)
    T = int(np.ceil(T_raw / NCORES)) * NCORES
    tpc = T // NCORES
    At = T * 128

    class_of_tile = []
    base = {}
    pos = 0
    for c in range(C):
        base[c] = pos
        class_of_tile += [c] * tiles_c[c]
        pos += tiles_c[c] * 128
    class_of_tile += [C - 1] * (T - T_raw)

    a_sc = (anchors / TEMP)                      # fold 1/temperature in
    ast = np.zeros((At, D), np.float32)
    col_of_orig = np.empty(A, np.int64)
    for c in range(C):
        idx = np.nonzero(labels == c)[0]
        ast[base[c]: base[c] + len(idx)] = a_sc[idx]
        col_of_orig[idx] = base[c] + np.arange(len(idx))

    astT = np.ascontiguousarray(ast.T)           # [256, At]
    protoT = np.ascontiguousarray(
        proto_mem.reshape(P, D).T)               # [256, 608]

    # ---- launch A ------------------------------------------------------
    nca = build_phase_a(tpc)
    a_maps = []
    for i in range(NCORES):
        sl = astT[:, i * tpc * 128: (i + 1) * tpc * 128]
        blob = np.concatenate(
            [sl[:128], sl[128:], protoT[:128], protoT[128:]],
            axis=1).astype(np.float32)
        a_maps.append({"blob": np.ascontiguousarray(blob)})
    resA = _run(nca, a_maps, _profile)

    dp = np.empty((At, P), np.float32)           # raw dots (already /T)
    for i in range(NCORES):
        o = resA.results[i]["dots"]              # [128, tpc*608]
        for tl in range(tpc):
            dp[(i * tpc + tl) * 128: (i * tpc + tl + 1) * 128] = \
                o[:, tl * P: (tl + 1) * P]

    # ---- logits_max on host (exact masked row max) ---------------------
    valid = proto_mask.reshape(P).astype(bool)
    m_all = np.max(np.where(valid[None, :], dp, np.float32(-np.inf)),
                   axis=1).astype(np.float32)    # [At]

    # ---- launch B ------------------------------------------------------
    mneg_mat = (-m_all).reshape(T, 128).T.astype(np.float32)  # [128, T]
    ncb = build_phase_b(T, class_of_tile)
    b_maps = []
    for i in range(NCORES):
        lsh = local_mem[:, i * NSH: (i + 1) * NSH, :]         # [19,256,256]
        # lstk[k][d, c*NSH+n] = lsh[c, n, k*128+d]
        lst = np.ascontiguousarray(
            lsh.transpose(2, 0, 1).reshape(D, C * NSH))        # [256, 19*256]
        blob = np.concatenate(
            [astT[:128], astT[128:], lst[:128], lst[128:], mneg_mat],
            axis=1).astype(np.float32)
        b_maps.append({"blob": np.ascontiguousarray(blob)})
    resB = _run(ncb, b_maps, _profile)

    negp = np.zeros((128, T), np.float32)
    emax = np.full((128, T), -np.inf, np.float32)
    for i in range(NCORES):
        o = resB.results[i]["out"]
        negp = negp + o[:, :T]
        emax = np.maximum(emax, o[:, T:])
    neg = negp.T.reshape(At)                     # [At]
    emaxv = emax.T.reshape(At)
    # restore IEEE overflow: any exp term that hit float32 max -> inf sum
    neg = np.where(emaxv >= FMAX, np.float32(np.inf), neg).astype(np.float32)

    # ---- final log-prob math on host (float32, reference semantics) ----
    cls = np.array(class_of_tile, np.int32).repeat(128)        # [At]
    ccol = (cls[:, None] * U + np.arange(U)[None, :])          # [At, 32]
    pos_dots = np.take_along_axis(dp, ccol, axis=1)            # [At, 32]
    vm = proto_mask[cls].astype(np.float32)                    # [At, 32]

    l = pos_dots - m_all[:, None]                              # logits
    e = np.exp(np.where(vm > 0, l, np.float32(-np.inf)))
    s = (e + neg[:, None]) + EPS
    lp = l - np.log(s)
    msum = np.sum(vm * lp, axis=1, dtype=np.float32)
    mlpp = msum / (np.sum(vm, axis=1, dtype=np.float32) + EPS)

    loss = np.mean(-mlpp[col_of_orig].astype(np.float32))
    return np.float32(loss)


# revision 19
# speedup vs baseline: 1.0046x; 1.0046x over previous
"""Trainium2 Bass kernel for nn_ContrastLossLocal (supervised-contrastive loss).

Strategy (8 NeuronCores, SPMD via run_bass_kernel_spmd):
  - Host sorts anchors by class label and pads each class block to a
    multiple of 128 rows; total padded tiles rounded to a multiple of 8.
  - Launch A (anchor tiles sharded over cores): dots of each core's anchor
    tiles vs all 608 prototypes (pure fp32 matmul), shipped to host.
    Host derives logits_max (masked row max, exact) from these dots.
  - Launch B (local memory sharded 8-way over the N=2048 axis, anchors
    replicated): per class, matmul of that class's anchor tiles vs the
    core's 256-row local shard, exp(dots - logits_max) with on-ACT
    accumulation -> per-anchor partial negative sums; also the row max of
    the exp values so the host can restore IEEE inf semantics even if the
    ACT table saturates on overflow.
  - Host: sums partials across cores, fixes overflow rows to +inf, and
    evaluates the final [4096, 32] log-prob/masked-mean in float32 numpy
    (bit-faithful to the reference's where/exp/log/0*inf semantics).

Every instruction is arranged to carry at most ONE semaphore wait (this
toolchain's walrus rejects multi-wait instructions): single blob load DMA,
single-engine writers per DMA'd tile, PSUM read by exactly one engine,
no scratch-slot reuse on the ACT/DVE pipe.
"""
import numpy as np

import concourse.bass as bass
import concourse.tile as tile
import concourse.mybir as mybir
from concourse.bass_utils import run_bass_kernel_spmd

F32 = mybir.dt.float32
AF = mybir.ActivationFunctionType
AX = mybir.AxisListType


def _install_drain_split():
    """This container's walrus rejects instructions with >1 semaphore wait.
    Tile's kernel-tail drain collects one wait per processor; split the
    surplus onto extra drain instructions (same guarantee, 1 wait each)."""
    from concourse.vector_clock import ScopedClock

    def patched(self, tick_clock, wait_clock):
        drain_inst = self.nc.sync.drain()
        wait_clock.add_sem_waits(
            drain_inst.ins, ScopedClock({None: tick_clock.global_clock}))
        si = drain_inst.ins.sync_info
        if si is not None:
            waits = list(si.on_wait)
            if len(waits) > 1:
                drain_inst.ins.sync_info = mybir.SyncInfo(
                    on_wait=[waits[0]], on_update=list(si.on_update))
                for w in waits[1:]:
                    extra = self.nc.sync.drain()
                    extra.ins.sync_info = mybir.SyncInfo(
                        on_wait=[w], on_update=[])
        self.nc.all_engine_barrier()
        assert self.sems is not None
        popped = self.nc._tile_sem_poison_stack.pop()
        assert popped is self._sem_poison
        self.nc.clear_and_free_semaphores(list(self.sems.allocated().values()))
        self.nc.all_engine_barrier()

    tile.TileContext._drain_and_barrier = patched


_install_drain_split()

NCORES = 8
C, U, N, D = 19, 32, 2048, 256
A = 4096
P = C * U            # 608
NSH = N // NCORES    # 256 local rows per class per core
TEMP = np.float32(0.1)
EPS = np.float32(1e-8)
FMAX = np.float32(np.finfo(np.float32).max)


# ---------------------------------------------------------------- launch A
def build_phase_a(tpc):
    """Per core: its tpc anchor tiles x all 608 protos -> raw dots out.

    Split loads (protos first, then per-tile anchor chunks) and per-tile
    stores so DMA overlaps PE/DVE work."""
    nc = bass.Bass()
    ncols = 2 * tpc * 128 + 2 * P            # astk0|astk1|ptk0|ptk1
    blob = nc.declare_dram_parameter("blob", [128, ncols], F32, isOutput=False)
    dots = nc.declare_dram_parameter("dots", [128, tpc * P], F32, isOutput=True)

    a0, a1 = 0, tpc * 128
    p0, p1 = 2 * tpc * 128, 2 * tpc * 128 + P

    with tile.TileContext(nc) as tc:
        with tc.tile_pool(name="sbuf", bufs=1) as pool, \
             tc.tile_pool(name="psum", bufs=2, space="PSUM") as pp:
            sb = pool.tile([128, ncols], F32)
            # 2 loads + 5 stores = 7 DMAs <= 8 queues: no queue reuse, so
            # every DMA carries at most its single data wait.
            nc.sync.dma_start(sb[:, p0: p0 + 2 * P], blob[:, p0: p0 + 2 * P])
            nc.sync.dma_start(sb[:, a0: a0 + 2 * tpc * 128],
                              blob[:, a0: a0 + 2 * tpc * 128])
            out = pool.tile([128, tpc * P], F32)

            # absorber matmuls accumulate into one never-read PSUM bank
            # (accumulation -> no bank-overlap serialization wait)
            dummy = pp.tile([128, 1], F32, tag="dummy")
            nc.tensor.matmul(dummy[:], sb[:, p0: p0 + 128], sb[:, p0: p0 + 1],
                             start=True, stop=False, skip_group_check=True)
            nc.tensor.matmul(dummy[:], sb[:, a0: a0 + 128], sb[:, a0: a0 + 1],
                             start=False, stop=True, skip_group_check=True)
            for t in range(tpc):
                psA = pp.tile([128, 320], F32, tag="psA")
                psB = pp.tile([128, 288], F32, tag="psB")
                for k, ak in ((0, a0), (1, a1)):
                    lhsT = sb[:, ak + t * 128: ak + (t + 1) * 128]
                    pk = p0 if k == 0 else p1
                    nc.tensor.matmul(psA[:], lhsT, sb[:, pk: pk + 320],
                                     start=(k == 0), stop=(k == 1))
                    nc.tensor.matmul(psB[:], lhsT, sb[:, pk + 320: pk + P],
                                     start=(k == 0), stop=(k == 1))
                nc.vector.tensor_scalar_add(out[:, t * P: t * P + 320],
                                            psA[:], 0.0)
                nc.vector.tensor_scalar_add(out[:, t * P + 320: (t + 1) * P],
                                            psB[:], 0.0)
                nc.sync.dma_start(dots[:, t * P: (t + 1) * P],
                                  out[:, t * P: (t + 1) * P])
    return nc


# ---------------------------------------------------------------- launch B
def build_phase_b(T, class_of_tile):
    """All T anchor tiles x this core's local shard (per-class 256 rows).

    Loads are split (mneg, per-class locals, per-quarter anchors) so DMA
    streams ahead of the PE; a standalone load_weights absorbs each new
    locals-chunk queue sem so no matmul ever needs two waits."""
    nc = bass.Bass()
    At = T * 128
    ncols = 2 * At + 2 * C * NSH + T     # ast(3 chunks, k0|k1 each)|lst|mneg
    blob = nc.declare_dram_parameter("blob", [128, ncols], F32, isOutput=False)
    outp = nc.declare_dram_parameter("out", [128, 2 * T], F32, isOutput=True)

    # anchor tiles in 3 chunks; within chunk x: [k0 cols | k1 cols]
    bnd = [0, (T + 2) // 3, 2 * (T + 2) // 3, T]
    cbase = [2 * 128 * bnd[x] for x in range(4)]

    def acol(t, k):
        x = 0 if t < bnd[1] else (1 if t < bnd[2] else 2)
        w = bnd[x + 1] - bnd[x]
        return cbase[x] + k * w * 128 + (t - bnd[x]) * 128

    l0 = 2 * At                          # class c: [l0+c*512, l0+(c+1)*512)
    mcol = 2 * At + 2 * C * NSH
    lgrp = [0, 7, 13, 19]                # locals in 3 group loads

    with tile.TileContext(nc) as tc:
        with tc.tile_pool(name="sbuf", bufs=1) as pool, \
             tc.tile_pool(name="psum", bufs=4, space="PSUM") as pp:
            sb = pool.tile([128, ncols], F32)
            # 7 loads + 1 store = 8 DMAs on 8 queues: no queue reuse.
            # Issue in use order: mneg, ast0, lst_g0, ast1, lst_g1, ast2,
            # lst_g2 (queues run concurrently; order still biases arrival).
            nc.sync.dma_start(sb[:, mcol: mcol + T], blob[:, mcol: mcol + T])
            lg = [l0 + lgrp[g] * 2 * NSH for g in range(4)]
            for g in range(3):
                nc.sync.dma_start(sb[:, cbase[g]: cbase[g + 1]],
                                  blob[:, cbase[g]: cbase[g + 1]])
                nc.sync.dma_start(sb[:, lg[g]: lg[g + 1]],
                                  blob[:, lg[g]: lg[g + 1]])

            # ACT absorber: observe the mneg DMA once so each exp only
            # needs the single PE wait (walrus allows 1 wait/instr).
            absorb = pool.tile([128, 1], F32)
            nc.scalar.copy(absorb[:], sb[:, mcol: mcol + 1])

            ebig = pool.tile([128, T * NSH], F32)     # exp scratch, no reuse
            negp = pool.tile([128, T], F32)           # ACT accum sums
            emax = pool.tile([128, T], F32)           # DVE row maxes
            outb = pool.tile([128, 2 * T], F32)       # DVE-written store buf

            # absorber matmuls accumulate into one never-read PSUM bank
            # (accumulation -> no bank-overlap serialization wait); one per
            # load chunk, placed at first use so PE streams behind the DMA.
            dummy = pp.tile([128, 1], F32, tag="dummy")
            seen_ast = set()
            seen_lst = set()
            ndum = 0

            def absorb_pe(col):
                nonlocal ndum
                nc.tensor.matmul(dummy[:], sb[:, col: col + 128],
                                 sb[:, col: col + 1],
                                 start=(ndum == 0), stop=(ndum == 5),
                                 skip_group_check=True)
                ndum += 1

            for t in range(T):
                c = class_of_tile[t]
                lc = l0 + c * 2 * NSH
                x = 0 if t < bnd[1] else (1 if t < bnd[2] else 2)
                if x not in seen_ast:
                    absorb_pe(acol(t, 0))
                    seen_ast.add(x)
                g = 0 if c < lgrp[1] else (1 if c < lgrp[2] else 2)
                if g not in seen_lst:
                    absorb_pe(lc)
                    seen_lst.add(g)
                ps = pp.tile([128, NSH], F32, tag="ps")
                for k in (0, 1):
                    nc.tensor.matmul(
                        ps[:],
                        sb[:, acol(t, k): acol(t, k) + 128],
                        sb[:, lc + k * NSH: lc + (k + 1) * NSH],
                        start=(k == 0), stop=(k == 1))
                esl = ebig[:, t * NSH: (t + 1) * NSH]
                nc.scalar.activation(esl, ps[:], AF.Exp,
                                     bias=sb[:, mcol + t: mcol + t + 1],
                                     accum_out=negp[:, t: t + 1])
                nc.vector.reduce_max(emax[:, t: t + 1], esl, axis=AX.X)

            nc.vector.tensor_scalar_add(outb[:, :T], negp[:], 0.0)
            nc.vector.tensor_scalar_add(outb[:, T:], emax[:], 0.0)
            nc.sync.dma_start(outp[:], outb[:])
    return nc


# ---------------------------------------------------------------- host glue
def _run(nc, in_maps, profile):
    """Run the SPMD program; if profiling, re-run warm and record wall ns."""
    import time
    res = run_bass_kernel_spmd(nc, in_maps, list(range(NCORES)), trace=False)
    if profile is not None:
        t0 = time.perf_counter()
        run_bass_kernel_spmd(nc, in_maps, list(range(NCORES)), trace=False)
        profile.append(int((time.perf_counter() - t0) * 1e9))
    return res


def kernel(anchors, anchor_labels, proto_mem, proto_mask, local_mem,
           _profile=None):
    anchors = np.asarray(anchors, np.float32)
    labels = np.asarray(anchor_labels, np.int32)
    proto_mem = np.asarray(proto_mem, np.float32)
    proto_mask = np.asarray(proto_mask, np.int32)
    local_mem = np.asarray(local_mem, np.float32)

    # ---- sort/pad anchors by class -------------------------------------
    counts = np.bincount(labels, minlength=C)
    tiles_c = [int(np.ceil(n / 128)) if n else 0 for n in counts]
    T_raw = sum(tiles_c)
    T = int(np.ceil(T_raw / NCORES)) * NCORES
    tpc = T // NCORES
    At = T * 128

    class_of_tile = []
    base = {}
    pos = 0
    for c in range(C):
        base[c] = pos
        class_of_tile += [c] * tiles_c[c]
        pos += tiles_c[c] * 128
    class_of_tile += [C - 1] * (T - T_raw)

    a_sc = (anchors / TEMP)                      # fold 1/temperature in
    ast = np.zeros((At, D), np.float32)
    col_of_orig = np.empty(A, np.int64)
    for c in range(C):
        idx = np.nonzero(labels == c)[0]
        ast[base[c]: base[c] + len(idx)] = a_sc[idx]
        col_of_orig[idx] = base[c] + np.arange(len(idx))

    astT = np.ascontiguousarray(ast.T)           # [256, At]
    protoT = np.ascontiguousarray(
        proto_mem.reshape(P, D).T)               # [256, 608]

    # ---- launch A ------------------------------------------------------
    nca = build_phase_a(tpc)
    a_maps = []
    for i in range(NCORES):
        sl = astT[:, i * tpc * 128: (i + 1) * tpc * 128]
        blob = np.concatenate(
            [sl[:128], sl[128:], protoT[:128], protoT[128:]],
            axis=1).astype(np.float32)
        a_maps.append({"blob": np.ascontiguousarray(blob)})
    resA = _run(nca, a_maps, _profile)

    dp = np.empty((At, P), np.float32)           # raw dots (already /T)
    for i in range(NCORES):
        o = resA.results[i]["dots"]              # [128, tpc*608]
        for tl in range(tpc):
            dp[(i * tpc + tl) * 128: (i * tpc + tl + 1) * 128] = \
                o[:, tl * P: (tl + 1) * P]

    # ---- logits_max on host (exact masked row max) ---------------------
    valid = proto_mask.reshape(P).astype(bool)
    m_all = np.max(np.where(valid[None, :], dp, np.float32(-np.inf)),
                   axis=1).astype(np.float32)    # [At]

    # ---- launch B ------------------------------------------------------
    mneg_mat = (-m_all).reshape(T, 128).T.astype(np.float32)  # [128, T]
    ncb = build_phase_b(T, class_of_tile)
    b_maps = []
    for i in range(NCORES):
        lsh = local_mem[:, i * NSH: (i + 1) * NSH, :]         # [19,256,256]
        full = lsh.transpose(2, 0, 1)                          # [256,19,256]
        # interleave per class: [c*512 + k*256 + n] = lsh[c, n, k*128+d]
        lsti = np.ascontiguousarray(
            np.stack([full[:128], full[128:]], axis=2)
            .reshape(128, C * 2 * NSH))
        # ast in 3 chunks, each [k0 cols | k1 cols], matching acol()
        bnd = [0, (T + 2) // 3, 2 * (T + 2) // 3, T]
        ast_parts = []
        for x in range(3):
            s, e = bnd[x] * 128, bnd[x + 1] * 128
            ast_parts += [astT[:128, s:e], astT[128:, s:e]]
        blob = np.concatenate(ast_parts + [lsti, mneg_mat],
                              axis=1).astype(np.float32)
        b_maps.append({"blob": np.ascontiguousarray(blob)})
    resB = _run(ncb, b_maps, _profile)

    negp = np.zeros((128, T), np.float32)
    emax = np.full((128, T), -np.inf, np.float32)
    for i in range(NCORES):
        o = resB.results[i]["out"]
        negp = negp + o[:, :T]
        emax = np.maximum(emax, o[:, T:])
    neg = negp.T.reshape(At)                     # [At]
    emaxv = emax.T.reshape(At)
    # restore IEEE overflow: any exp term that hit float32 max -> inf sum
    neg = np.where(emaxv >= FMAX, np.float32(np.inf), neg).astype(np.float32)

    # ---- final log-prob math on host (float32, reference semantics) ----
    cls = np.array(class_of_tile, np.int32).repeat(128)        # [At]
    ccol = (cls[:, None] * U + np.arange(U)[None, :])          # [At, 32]
    pos_dots = np.take_along_axis(dp, ccol, axis=1)            # [At, 32]
    vm = proto_mask[cls].astype(np.float32)                    # [At, 32]

    l = pos_dots - m_all[:, None]                              # logits
    e = np.exp(np.where(vm > 0, l, np.float32(-np.inf)))
    s = (e + neg[:, None]) + EPS
    lp = l - np.log(s)
    msum = np.sum(vm * lp, axis=1, dtype=np.float32)
    mlpp = msum / (np.sum(vm, axis=1, dtype=np.float32) + EPS)

    loss = np.mean(-mlpp[col_of_orig].astype(np.float32))
    return np.float32(loss)
